# revision 4
# baseline (speedup 1.0000x reference)
"""DeepFloorFusedMixer Trainium2 kernel.

Math (per batch b):
    normed = rms_norm(state)
    q = phi(normed @ Wq^T); k = phi(normed @ Wk^T); v = normed @ Wv^T
    acc = 0.95*acc_in + (k^T v)/S
    out = state + (q @ acc)/sqrt(D) @ Wo^T

Algebraic restructure used on device:
    G^T = (nb^T phi_k)^T  accumulated over sequence (per core: its half)
    kv^T = (Wv^T/S)^T-contract G^T      (v never materialized)
    accT = 0.95*accT_in + kvT           (transposed accumulator)
    M = accT^T-contract (Wo^T/sqrt(D))  -> out = state + qT^T-contract M

Sharding: core c handles batch c//2, sequence half c%2 (4096 rows).
The per-pair [D,D] G^T partial sums are combined with a 2-core AllReduce.
All matmuls run in bf16 with fp32 PSUM accumulation.
"""

import sys
import numpy as np

sys.path.insert(0, "/opt/trn_rl_repo")

import ml_dtypes  # noqa: E402
import concourse.bass as bass  # noqa: E402
import concourse.bacc as bacc  # noqa: E402
import concourse.tile as tile  # noqa: E402
import concourse.mybir as mybir  # noqa: E402
from concourse import bass_utils, masks  # noqa: E402

F32 = mybir.dt.float32
BF16 = mybir.dt.bfloat16
NPBF16 = ml_dtypes.bfloat16

B, S, D = 4, 8192, 1024
N_CORES = 8
S_LOC = S // 2          # rows per core
P = 128                 # partitions
NT = D // P             # 8 feature tiles
N_BLK = 4               # blocks per core
BLK_ST = S_LOC // (N_BLK * P)   # 8 s_tiles per block
BLK_S = BLK_ST * P      # 1024 rows per block
DECAY = 0.95
EPS = 1e-6

# module-level cache: build+compile once per process
_CACHE = {}

# set by callers that want profiling (see test.py)
TRACE = False
LAST_EXEC_NS = None
LAST_RESULTS = None


def _build():
    nc = bacc.Bacc("TRN2", target_bir_lowering=False, debug=False,
                   num_devices=N_CORES)

    x_d = nc.dram_tensor("x", [S_LOC, D], F32, kind="ExternalInput")
    wk_d = nc.dram_tensor("wk_t", [D, D], BF16, kind="ExternalInput")
    wq_d = nc.dram_tensor("wq_t", [D, D], BF16, kind="ExternalInput")
    wv_d = nc.dram_tensor("wv_t", [D, D], BF16, kind="ExternalInput")
    wo_d = nc.dram_tensor("wo_t", [D, D], BF16, kind="ExternalInput")
    acc_d = nc.dram_tensor("acc_t", [D, D], F32, kind="ExternalInput")
    out_d = nc.dram_tensor("out", [S_LOC, D], F32, kind="ExternalOutput")
    acco_d = nc.dram_tensor("acc_o", [D, D], F32, kind="ExternalOutput")

    x_v = x_d.ap().rearrange("(n p) d -> n p d", p=P)      # [32, 128, 1024]
    out_v = out_d.ap().rearrange("(n p) d -> n p d", p=P)
    acc_v = acc_d.ap().rearrange("(n p) d -> n p d", p=P)  # [8, 128, 1024]
    acco_v = acco_d.ap().rearrange("(n p) d -> n p d", p=P)
    # weights viewed as [tile, 128, 1024]
    wk_v = wk_d.ap().rearrange("(n p) d -> p n d", p=P)    # [128, 8, 1024]
    wq_v = wq_d.ap().rearrange("(n p) d -> p n d", p=P)
    wv_v = wv_d.ap().rearrange("(n p) d -> p n d", p=P)
    wo_v = wo_d.ap().rearrange("(n p) d -> p n d", p=P)

    with tile.TileContext(nc) as tc:
        with (
            tc.tile_pool(name="const", bufs=1) as constp,
            tc.tile_pool(name="wpool", bufs=2) as wpool,
            tc.tile_pool(name="gpool", bufs=1) as gpool,
            tc.tile_pool(name="btpool", bufs=4) as btpool,
            tc.tile_pool(name="xpool", bufs=3) as xpool,
            tc.tile_pool(name="opool", bufs=2) as opool,
            tc.tile_pool(name="sqpool", bufs=2) as sqpool,
            tc.tile_pool(name="stpool", bufs=4) as stpool,
            tc.tile_pool(name="phpool", bufs=3) as phpool,
            tc.tile_pool(name="qtpool", bufs=2) as qtpool,
            tc.tile_pool(name="acpool", bufs=2) as acpool,
            tc.tile_pool(name="perm", bufs=1) as permp,
            tc.tile_pool(name="mmps", bufs=3, space="PSUM") as mmps,
            tc.tile_pool(name="trps", bufs=2, space="PSUM") as trps,
            tc.tile_pool(name="dram", bufs=1, space="DRAM") as dramp,
        ):
            ident = constp.tile([P, P], BF16)
            masks.make_identity(nc, ident[:])
            eps_t = constp.tile([P, 1], F32)
            nc.gpsimd.memset(eps_t[:], EPS)

            wk_sb = wpool.tile([P, NT, D], BF16, name="wk_sb", tag="w")
            wq_sb = wpool.tile([P, NT, D], BF16, name="wq_sb", tag="w")
            nc.sync.dma_start(wk_sb[:], wk_v)
            nc.sync.dma_start(wq_sb[:], wq_v)

            gt_sbuf = gpool.tile([P, NT, D], BF16, name="gt_sbuf", tag="gt")
            acct_bf = permp.tile([P, NT, D], BF16, name="acct_bf", tag="acct")
            m_sb = permp.tile([P, NT, D], BF16, name="m_sb", tag="m")

            qt_spill = dramp.tile([N_BLK, NT, P, BLK_S], BF16, name="qt_spill", tag="qt_spill")
            gt_loc = dramp.tile([P, NT, D], BF16, name="gt_loc", tag="gt_loc")
            gt_red = dramp.tile([P, NT, D], BF16, name="gt_red", tag="gt_red")

            # ---------------- block loop ----------------
            for blk in range(N_BLK):
                nb_blk = btpool.tile([P, BLK_ST, D], BF16, name=f"nb{blk}", tag="bt")
                nbt_blk = btpool.tile([P, NT, BLK_S], BF16, name=f"nbt{blk}", tag="bt")
                phik_blk = btpool.tile([P, BLK_ST, D], BF16, name=f"phik{blk}", tag="bt")

                for sl in range(BLK_ST):
                    st = blk * BLK_ST + sl
                    # --- rmsnorm ---
                    x_t = xpool.tile([P, D], F32, name="x_t", tag="x_t")
                    nc.sync.dma_start(x_t[:], x_v[st])
                    sq = sqpool.tile([P, D], BF16, name="sq", tag="sq")
                    ss = stpool.tile([P, 1], F32, name="ss", tag="ss")
                    nc.scalar.activation(
                        sq[:], x_t[:], mybir.ActivationFunctionType.Square,
                        accum_out=ss[:],
                    )
                    sr = stpool.tile([P, 1], F32, name="sr", tag="sr")
                    nc.scalar.activation(
                        sr[:], ss[:], mybir.ActivationFunctionType.Sqrt,
                        bias=eps_t[:], scale=1.0 / D,
                    )
                    rstd = stpool.tile([P, 1], F32, name="rstd", tag="rstd")
                    nc.vector.reciprocal(rstd[:], sr[:])
                    nc.vector.tensor_scalar_mul(
                        nb_blk[:, sl, :], x_t[:], rstd[:])

                    # --- transpose nb tile -> nbt (one psum group per bank) ---
                    tr = trps.tile([P, D], BF16, name="tr", tag="tr")
                    for dt in range(NT):
                        nc.tensor.matmul(
                            tr[:, dt * P:(dt + 1) * P],
                            nb_blk[:, sl, dt * P:(dt + 1) * P],
                            ident[:],
                            is_transpose=True,
                            start=(dt == 0), stop=(dt == NT - 1),
                        )
                    # scatter psum -> nbt_blk[:, :, sl*P : (sl+1)*P]
                    nc.scalar.copy(
                        nbt_blk[:, :, sl * P:(sl + 1) * P],
                        tr[:].rearrange("p (n f) -> p n f", f=P),
                    )

                    # --- k projection ---
                    kp = mmps.tile([P, D], F32, name="kp", tag="mm")
                    for dt in range(NT):
                        for hf in range(2):
                            nc.tensor.matmul(
                                kp[:, hf * 512:(hf + 1) * 512],
                                nbt_blk[:, dt, sl * P:(sl + 1) * P],
                                wk_sb[:, dt, hf * 512:(hf + 1) * 512],
                                start=(dt == 0), stop=(dt == NT - 1),
                            )
                    # --- phi(k) = max(x+1, min(exp(x), 1)) ---
                    ex = phpool.tile([P, D], BF16, name="ex", tag="ex")
                    nc.scalar.activation(
                        ex[:], kp[:], mybir.ActivationFunctionType.Exp)
                    xp1 = phpool.tile([P, D], BF16, name="xp1", tag="xp1")
                    nc.scalar.activation(
                        xp1[:], kp[:], mybir.ActivationFunctionType.Identity,
                        bias=1.0)
                    nc.vector.tensor_scalar_min(ex[:], ex[:], 1.0)
                    nc.vector.tensor_tensor(
                        phik_blk[:, sl, :], ex[:], xp1[:],
                        op=mybir.AluOpType.max)

                # --- G^T accumulation: GT[g,d] += sum_s nb[s,g] phik[s,d] ---
                for gt in range(NT):
                    gp = mmps.tile([P, D], F32, name="gp", tag="mm")
                    for sl in range(BLK_ST):
                        for hf in range(2):
                            nc.tensor.matmul(
                                gp[:, hf * 512:(hf + 1) * 512],
                                nb_blk[:, sl, gt * P:(gt + 1) * P],
                                phik_blk[:, sl, hf * 512:(hf + 1) * 512],
                                start=(sl == 0), stop=(sl == BLK_ST - 1),
                            )
                    if blk == 0:
                        nc.vector.tensor_copy(gt_sbuf[:, gt, :], gp[:])
                    else:
                        nc.vector.tensor_tensor(
                            gt_sbuf[:, gt, :], gp[:], gt_sbuf[:, gt, :],
                            op=mybir.AluOpType.add)

                # --- qT production for this block (spilled to DRAM) ---
                for et in range(NT):
                    qp = mmps.tile([P, BLK_S], F32, name="qp", tag="mm")
                    for dt in range(NT):
                        for hf in range(2):
                            nc.tensor.matmul(
                                qp[:, hf * 512:(hf + 1) * 512],
                                wq_sb[:, dt, et * P:(et + 1) * P],
                                nbt_blk[:, dt, hf * 512:(hf + 1) * 512],
                                start=(dt == 0), stop=(dt == NT - 1),
                            )
                    exq = phpool.tile([P, BLK_S], BF16, name="exq", tag="ex")
                    nc.scalar.activation(
                        exq[:], qp[:], mybir.ActivationFunctionType.Exp)
                    xp1q = phpool.tile([P, BLK_S], BF16, name="xp1q", tag="xp1")
                    nc.scalar.activation(
                        xp1q[:], qp[:], mybir.ActivationFunctionType.Identity,
                        bias=1.0)
                    nc.vector.tensor_scalar_min(exq[:], exq[:], 1.0)
                    qt_t = qtpool.tile([P, BLK_S], BF16, name="qt_t", tag="qt")
                    nc.vector.tensor_tensor(
                        qt_t[:], exq[:], xp1q[:], op=mybir.AluOpType.max)
                    nc.sync.dma_start(qt_spill[blk, et], qt_t[:])

            # ---------------- all-reduce of G^T over the pair ----------------
            nc.sync.dma_start(gt_loc[:], gt_sbuf[:])
            nc.gpsimd.collective_compute(
                "AllReduce",
                mybir.AluOpType.add,
                replica_groups=[[0, 1], [2, 3], [4, 5], [6, 7]],
                ins=[gt_loc[:].opt()],
                outs=[gt_red[:].opt()],
            )
            nc.sync.dma_start(gt_sbuf[:], gt_red[:])

            # ---------------- kv^T and accT ----------------
            wv_sb = wpool.tile([P, NT, D], BF16, name="wv_sb", tag="w")
            nc.sync.dma_start(wv_sb[:], wv_v)
            for et in range(NT):
                kvp = mmps.tile([P, D], F32, name="kvp", tag="mm")
                for gt in range(NT):
                    for hf in range(2):
                        nc.tensor.matmul(
                            kvp[:, hf * 512:(hf + 1) * 512],
                            wv_sb[:, gt, et * P:(et + 1) * P],
                            gt_sbuf[:, gt, hf * 512:(hf + 1) * 512],
                            start=(gt == 0), stop=(gt == NT - 1),
                        )
                a_in = acpool.tile([P, D], F32, name="a_in", tag="a_in")
                nc.sync.dma_start(a_in[:], acc_v[et])
                nc.vector.tensor_scalar_mul(a_in[:], a_in[:], DECAY)
                a_new = acpool.tile([P, D], F32, name="a_new", tag="a_new")
                nc.vector.tensor_tensor(
                    a_new[:], kvp[:], a_in[:], op=mybir.AluOpType.add)
                nc.sync.dma_start(acco_v[et], a_new[:])
                nc.vector.tensor_copy(acct_bf[:, et, :], a_new[:])

            # ---------------- M = accT.T-contract WoT ----------------
            wo_sb = wpool.tile([P, NT, D], BF16, name="wo_sb", tag="w")
            nc.sync.dma_start(wo_sb[:], wo_v)
            for dt in range(NT):
                mp = mmps.tile([P, D], F32, name="mp", tag="mm")
                for et in range(NT):
                    for hf in range(2):
                        nc.tensor.matmul(
                            mp[:, hf * 512:(hf + 1) * 512],
                            acct_bf[:, et, dt * P:(dt + 1) * P],
                            wo_sb[:, et, hf * 512:(hf + 1) * 512],
                            start=(et == 0), stop=(et == NT - 1),
                        )
                nc.scalar.copy(m_sb[:, dt, :], mp[:])

            # ---------------- out = x + qT.T-contract M ----------------
            for blk in range(N_BLK):
                qt_blk = btpool.tile([P, NT, BLK_S], BF16, name=f"qtb{blk}", tag="bt")
                nc.sync.dma_start(qt_blk[:], qt_spill[blk].rearrange("n p f -> p n f"))
                for sl in range(BLK_ST):
                    st = blk * BLK_ST + sl
                    xo_t = xpool.tile([P, D], F32, name="xo_t", tag="x_t")
                    nc.sync.dma_start(xo_t[:], x_v[st])
                    op_ps = mmps.tile([P, D], F32, name="op_ps", tag="mm")
                    for hf in range(2):
                        for dt in range(NT):
                            nc.tensor.matmul(
                                op_ps[:, hf * 512:(hf + 1) * 512],
                                qt_blk[:, dt, sl * P:(sl + 1) * P],
                                m_sb[:, dt, hf * 512:(hf + 1) * 512],
                                start=(dt == 0), stop=(dt == NT - 1),
                            )
                    o_t = opool.tile([P, D], F32, name="o_t", tag="o_t")
                    nc.vector.tensor_tensor(
                        o_t[:], op_ps[:], xo_t[:], op=mybir.AluOpType.add)
                    nc.sync.dma_start(out_v[st], o_t[:])

    nc.compile()
    return nc


def _get_nc():
    if "nc" not in _CACHE:
        _CACHE["nc"] = _build()
    return _CACHE["nc"]


def kernel(state, accumulator, Wq, Wk, Wv, Wo):
    global LAST_EXEC_NS, LAST_RESULTS
    state = np.ascontiguousarray(np.asarray(state, dtype=np.float32))
    accumulator = np.ascontiguousarray(np.asarray(accumulator, dtype=np.float32))

    wq_t = np.ascontiguousarray(np.asarray(Wq, np.float32).T).astype(NPBF16)
    wk_t = np.ascontiguousarray(np.asarray(Wk, np.float32).T).astype(NPBF16)
    wv_t = (np.ascontiguousarray(np.asarray(Wv, np.float32).T) / S).astype(NPBF16)
    wo_t = (np.ascontiguousarray(np.asarray(Wo, np.float32).T)
            / np.sqrt(D)).astype(NPBF16)

    in_maps = []
    for c in range(N_CORES):
        b, h = c // 2, c % 2
        in_maps.append({
            "x": np.ascontiguousarray(state[b, h * S_LOC:(h + 1) * S_LOC]),
            "wq_t": wq_t, "wk_t": wk_t, "wv_t": wv_t, "wo_t": wo_t,
            "acc_t": np.ascontiguousarray(accumulator[b].T),
        })

    nc = _get_nc()
    res = bass_utils.run_bass_kernel_spmd(
        nc, in_maps, core_ids=list(range(N_CORES)), trace=TRACE)
    LAST_EXEC_NS = res.exec_time_ns
    LAST_RESULTS = res

    out = np.empty((B, S, D), dtype=np.float32)
    acc = np.empty((B, D, D), dtype=np.float32)
    for c in range(N_CORES):
        b, h = c // 2, c % 2
        out[b, h * S_LOC:(h + 1) * S_LOC] = res.results[c]["out"]
        if h == 0:
            acc[b] = res.results[c]["acc_o"].T
    return out, acc


# revision 6
# speedup vs baseline: 1.0273x; 1.0273x over previous
"""DeepFloorFusedMixer Trainium2 kernel.

Math (per batch b):
    normed = rms_norm(state)
    q = phi(normed @ Wq^T); k = phi(normed @ Wk^T); v = normed @ Wv^T
    acc = 0.95*acc_in + (k^T v)/S
    out = state + (q @ acc)/sqrt(D) @ Wo^T

Algebraic restructure used on device:
    G^T = (nb^T phi_k)^T  accumulated over sequence (per core: its half)
    kv^T = (Wv^T/S)^T-contract G^T      (v never materialized)
    accT = 0.95*accT_in + kvT           (transposed accumulator)
    M = accT^T-contract (Wo^T/sqrt(D))  -> out = state + qT^T-contract M

Sharding: core c handles batch c//2, sequence half c%2 (4096 rows).
The per-pair [D,D] G^T partial sums are combined with a 2-core AllReduce.
All matmuls run in bf16 with fp32 PSUM accumulation.
"""

import sys
import numpy as np

sys.path.insert(0, "/opt/trn_rl_repo")

import ml_dtypes  # noqa: E402
import concourse.bass as bass  # noqa: E402
import concourse.bacc as bacc  # noqa: E402
import concourse.tile as tile  # noqa: E402
import concourse.mybir as mybir  # noqa: E402
from concourse import bass_utils, masks  # noqa: E402

F32 = mybir.dt.float32
BF16 = mybir.dt.bfloat16
NPBF16 = ml_dtypes.bfloat16

B, S, D = 4, 8192, 1024
N_CORES = 8
S_LOC = S // 2          # rows per core
P = 128                 # partitions
NT = D // P             # 8 feature tiles
N_BLK = 4               # blocks per core
BLK_ST = S_LOC // (N_BLK * P)   # 8 s_tiles per block
BLK_S = BLK_ST * P      # 1024 rows per block
DECAY = 0.95
EPS = 1e-6

# module-level cache: build+compile once per process
_CACHE = {}

# set by callers that want profiling (see test.py)
TRACE = False
LAST_EXEC_NS = None
LAST_RESULTS = None


def _build():
    nc = bacc.Bacc("TRN2", target_bir_lowering=False, debug=False,
                   num_devices=N_CORES)

    x_d = nc.dram_tensor("x", [S_LOC, D], F32, kind="ExternalInput")
    wk_d = nc.dram_tensor("wk_t", [D, D], BF16, kind="ExternalInput")
    wq_d = nc.dram_tensor("wq_t", [D, D], BF16, kind="ExternalInput")
    wv_d = nc.dram_tensor("wv_t", [D, D], BF16, kind="ExternalInput")
    wo_d = nc.dram_tensor("wo_t", [D, D], BF16, kind="ExternalInput")
    acc_d = nc.dram_tensor("acc_t", [D, D], F32, kind="ExternalInput")
    out_d = nc.dram_tensor("out", [S_LOC, D], F32, kind="ExternalOutput")
    acco_d = nc.dram_tensor("acc_o", [D, D], F32, kind="ExternalOutput")

    x_v = x_d.ap().rearrange("(n p) d -> n p d", p=P)      # [32, 128, 1024]
    out_v = out_d.ap().rearrange("(n p) d -> n p d", p=P)
    acc_v = acc_d.ap().rearrange("(n p) d -> n p d", p=P)  # [8, 128, 1024]
    acco_v = acco_d.ap().rearrange("(n p) d -> n p d", p=P)
    # weights viewed as [tile, 128, 1024]
    wk_v = wk_d.ap().rearrange("(n p) d -> p n d", p=P)    # [128, 8, 1024]
    wq_v = wq_d.ap().rearrange("(n p) d -> p n d", p=P)
    wv_v = wv_d.ap().rearrange("(n p) d -> p n d", p=P)
    wo_v = wo_d.ap().rearrange("(n p) d -> p n d", p=P)

    with tile.TileContext(nc) as tc:
        with (
            tc.tile_pool(name="const", bufs=1) as constp,
            tc.tile_pool(name="wpool", bufs=2) as wpool,
            tc.tile_pool(name="gpool", bufs=1) as gpool,
            tc.tile_pool(name="btpool", bufs=4) as btpool,
            tc.tile_pool(name="xpool", bufs=3) as xpool,
            tc.tile_pool(name="opool", bufs=2) as opool,
            tc.tile_pool(name="sqpool", bufs=2) as sqpool,
            tc.tile_pool(name="stpool", bufs=4) as stpool,
            tc.tile_pool(name="phpool", bufs=3) as phpool,
            tc.tile_pool(name="qtpool", bufs=2) as qtpool,
            tc.tile_pool(name="acpool", bufs=2) as acpool,
            tc.tile_pool(name="perm", bufs=1) as permp,
            tc.tile_pool(name="mmps", bufs=3, space="PSUM") as mmps,
            tc.tile_pool(name="trps", bufs=2, space="PSUM") as trps,
            tc.tile_pool(name="dram", bufs=1, space="DRAM") as dramp,
        ):
            ident = constp.tile([P, P], BF16)
            masks.make_identity(nc, ident[:])
            eps_t = constp.tile([P, 1], F32)
            nc.gpsimd.memset(eps_t[:], EPS)

            wk_sb = wpool.tile([P, NT, D], BF16, name="wk_sb", tag="w")
            wq_sb = wpool.tile([P, NT, D], BF16, name="wq_sb", tag="w")
            nc.sync.dma_start(wk_sb[:], wk_v)
            nc.sync.dma_start(wq_sb[:], wq_v)

            gt_sbuf = gpool.tile([P, NT, D], BF16, name="gt_sbuf", tag="gt")
            acct_bf = permp.tile([P, NT, D], BF16, name="acct_bf", tag="acct")
            m_sb = permp.tile([P, NT, D], BF16, name="m_sb", tag="m")

            qt_spill = dramp.tile([N_BLK, NT, P, BLK_S], BF16, name="qt_spill", tag="qt_spill")
            gt_loc = dramp.tile([P, NT, D], BF16, name="gt_loc", tag="gt_loc")
            ag_out = dramp.tile([2, P, NT, D], BF16, name="ag_out", tag="ag_out")

            # ---------------- block loop ----------------
            for blk in range(N_BLK):
                nb_blk = btpool.tile([P, BLK_ST, D], BF16, name=f"nb{blk}", tag="bt")
                nbt_blk = btpool.tile([P, NT, BLK_S], BF16, name=f"nbt{blk}", tag="bt")
                phik_blk = btpool.tile([P, BLK_ST, D], BF16, name=f"phik{blk}", tag="bt")

                for sl in range(BLK_ST):
                    st = blk * BLK_ST + sl
                    # --- rmsnorm ---
                    x_t = xpool.tile([P, D], F32, name="x_t", tag="x_t")
                    nc.sync.dma_start(x_t[:], x_v[st])
                    sq = sqpool.tile([P, D], BF16, name="sq", tag="sq")
                    ss = stpool.tile([P, 1], F32, name="ss", tag="ss")
                    nc.scalar.activation(
                        sq[:], x_t[:], mybir.ActivationFunctionType.Square,
                        accum_out=ss[:],
                    )
                    sr = stpool.tile([P, 1], F32, name="sr", tag="sr")
                    nc.scalar.activation(
                        sr[:], ss[:], mybir.ActivationFunctionType.Sqrt,
                        bias=eps_t[:], scale=1.0 / D,
                    )
                    rstd = stpool.tile([P, 1], F32, name="rstd", tag="rstd")
                    nc.vector.reciprocal(rstd[:], sr[:])
                    nc.vector.tensor_scalar_mul(
                        nb_blk[:, sl, :], x_t[:], rstd[:])

                    # --- transpose nb tile -> nbt (one psum group per bank) ---
                    tr = trps.tile([P, D], BF16, name="tr", tag="tr")
                    for dt in range(NT):
                        nc.tensor.matmul(
                            tr[:, dt * P:(dt + 1) * P],
                            nb_blk[:, sl, dt * P:(dt + 1) * P],
                            ident[:],
                            is_transpose=True,
                            start=(dt == 0), stop=(dt == NT - 1),
                        )
                    # scatter psum -> nbt_blk[:, :, sl*P : (sl+1)*P]
                    nc.scalar.copy(
                        nbt_blk[:, :, sl * P:(sl + 1) * P],
                        tr[:].rearrange("p (n f) -> p n f", f=P),
                    )

                    # --- k projection ---
                    kp = mmps.tile([P, D], F32, name="kp", tag="mm")
                    for dt in range(NT):
                        for hf in range(2):
                            nc.tensor.matmul(
                                kp[:, hf * 512:(hf + 1) * 512],
                                nbt_blk[:, dt, sl * P:(sl + 1) * P],
                                wk_sb[:, dt, hf * 512:(hf + 1) * 512],
                                start=(dt == 0), stop=(dt == NT - 1),
                            )
                    # --- phi(k) = max(x+1, min(exp(x), 1)) ---
                    ex = phpool.tile([P, D], BF16, name="ex", tag="ex")
                    nc.scalar.activation(
                        ex[:], kp[:], mybir.ActivationFunctionType.Exp)
                    xp1 = phpool.tile([P, D], BF16, name="xp1", tag="xp1")
                    nc.vector.tensor_scalar_add(xp1[:], kp[:], 1.0)
                    nc.vector.tensor_scalar_min(ex[:], ex[:], 1.0)
                    nc.vector.tensor_tensor(
                        phik_blk[:, sl, :], ex[:], xp1[:],
                        op=mybir.AluOpType.max)

                # --- G^T accumulation: GT[g,d] += sum_s nb[s,g] phik[s,d] ---
                for gt in range(NT):
                    gp = mmps.tile([P, D], F32, name="gp", tag="mm")
                    for sl in range(BLK_ST):
                        for hf in range(2):
                            nc.tensor.matmul(
                                gp[:, hf * 512:(hf + 1) * 512],
                                nb_blk[:, sl, gt * P:(gt + 1) * P],
                                phik_blk[:, sl, hf * 512:(hf + 1) * 512],
                                start=(sl == 0), stop=(sl == BLK_ST - 1),
                            )
                    if blk == 0:
                        nc.vector.tensor_copy(gt_sbuf[:, gt, :], gp[:])
                    else:
                        nc.vector.tensor_tensor(
                            gt_sbuf[:, gt, :], gp[:], gt_sbuf[:, gt, :],
                            op=mybir.AluOpType.add)
                    if blk == N_BLK - 1:
                        nc.sync.dma_start(gt_loc[:, gt, :], gt_sbuf[:, gt, :])

                # --- qT production (deferred for the last block so it
                #     overlaps the collective) ---
                def emit_qt(qblk, qnbt):
                    for et in range(NT):
                        qp = mmps.tile([P, BLK_S], F32, name="qp", tag="mm")
                        for dt in range(NT):
                            for hf in range(2):
                                nc.tensor.matmul(
                                    qp[:, hf * 512:(hf + 1) * 512],
                                    wq_sb[:, dt, et * P:(et + 1) * P],
                                    qnbt[:, dt, hf * 512:(hf + 1) * 512],
                                    start=(dt == 0), stop=(dt == NT - 1),
                                )
                        exq = phpool.tile([P, BLK_S], BF16, name="exq", tag="ex")
                        nc.scalar.activation(
                            exq[:], qp[:], mybir.ActivationFunctionType.Exp)
                        xp1q = phpool.tile([P, BLK_S], BF16, name="xp1q", tag="xp1")
                        nc.vector.tensor_scalar_add(xp1q[:], qp[:], 1.0)
                        nc.vector.tensor_scalar_min(exq[:], exq[:], 1.0)
                        qt_t = qtpool.tile([P, BLK_S], BF16, name="qt_t", tag="qt")
                        nc.vector.tensor_tensor(
                            qt_t[:], exq[:], xp1q[:], op=mybir.AluOpType.max)
                        nc.sync.dma_start(qt_spill[qblk, et], qt_t[:])

                if blk < N_BLK - 1:
                    emit_qt(blk, nbt_blk)
                else:
                    last_nbt = nbt_blk

            # ------- pairwise exchange of G^T (AllGather + local sum) -------
            nc.gpsimd.collective_compute(
                "AllGather",
                mybir.AluOpType.bypass,
                replica_groups=[[0, 1], [2, 3], [4, 5], [6, 7]],
                ins=[gt_loc[:].opt()],
                outs=[ag_out[:].opt()],
            )
            # deferred qT of the last block runs on PE while the collective
            # is in flight
            emit_qt(N_BLK - 1, last_nbt)
            ag0 = btpool.tile([P, NT, D], BF16, name="ag0", tag="bt")
            ag1 = btpool.tile([P, NT, D], BF16, name="ag1", tag="bt")
            nc.sync.dma_start(ag0[:], ag_out[0])
            nc.sync.dma_start(ag1[:], ag_out[1])
            nc.vector.tensor_tensor(
                gt_sbuf[:], ag0[:], ag1[:], op=mybir.AluOpType.add)

            # ---------------- kv^T and accT ----------------
            wv_sb = wpool.tile([P, NT, D], BF16, name="wv_sb", tag="w")
            nc.sync.dma_start(wv_sb[:], wv_v)
            for et in range(NT):
                kvp = mmps.tile([P, D], F32, name="kvp", tag="mm")
                for gt in range(NT):
                    for hf in range(2):
                        nc.tensor.matmul(
                            kvp[:, hf * 512:(hf + 1) * 512],
                            wv_sb[:, gt, et * P:(et + 1) * P],
                            gt_sbuf[:, gt, hf * 512:(hf + 1) * 512],
                            start=(gt == 0), stop=(gt == NT - 1),
                        )
                a_in = acpool.tile([P, D], F32, name="a_in", tag="a_in")
                nc.sync.dma_start(a_in[:], acc_v[et])
                nc.vector.tensor_scalar_mul(a_in[:], a_in[:], DECAY)
                a_new = acpool.tile([P, D], F32, name="a_new", tag="a_new")
                nc.vector.tensor_tensor(
                    a_new[:], kvp[:], a_in[:], op=mybir.AluOpType.add)
                nc.sync.dma_start(acco_v[et], a_new[:])
                nc.vector.tensor_copy(acct_bf[:, et, :], a_new[:])

            # ---------------- M = accT.T-contract WoT ----------------
            wo_sb = wpool.tile([P, NT, D], BF16, name="wo_sb", tag="w")
            nc.sync.dma_start(wo_sb[:], wo_v)
            for dt in range(NT):
                mp = mmps.tile([P, D], F32, name="mp", tag="mm")
                for et in range(NT):
                    for hf in range(2):
                        nc.tensor.matmul(
                            mp[:, hf * 512:(hf + 1) * 512],
                            acct_bf[:, et, dt * P:(dt + 1) * P],
                            wo_sb[:, et, hf * 512:(hf + 1) * 512],
                            start=(et == 0), stop=(et == NT - 1),
                        )
                nc.scalar.copy(m_sb[:, dt, :], mp[:])

            # ---------------- out = x + qT.T-contract M ----------------
            for blk in range(N_BLK):
                qt_blk = btpool.tile([P, NT, BLK_S], BF16, name=f"qtb{blk}", tag="bt")
                nc.sync.dma_start(qt_blk[:], qt_spill[blk].rearrange("n p f -> p n f"))
                for sl in range(BLK_ST):
                    st = blk * BLK_ST + sl
                    xo_t = xpool.tile([P, D], F32, name="xo_t", tag="x_t")
                    nc.sync.dma_start(xo_t[:], x_v[st])
                    op_ps = mmps.tile([P, D], F32, name="op_ps", tag="mm")
                    for hf in range(2):
                        for dt in range(NT):
                            nc.tensor.matmul(
                                op_ps[:, hf * 512:(hf + 1) * 512],
                                qt_blk[:, dt, sl * P:(sl + 1) * P],
                                m_sb[:, dt, hf * 512:(hf + 1) * 512],
                                start=(dt == 0), stop=(dt == NT - 1),
                            )
                    o_t = opool.tile([P, D], F32, name="o_t", tag="o_t")
                    nc.vector.tensor_tensor(
                        o_t[:], op_ps[:], xo_t[:], op=mybir.AluOpType.add)
                    nc.sync.dma_start(out_v[st], o_t[:])

    nc.compile()
    return nc


def _get_nc():
    if "nc" not in _CACHE:
        _CACHE["nc"] = _build()
    return _CACHE["nc"]


def kernel(state, accumulator, Wq, Wk, Wv, Wo):
    global LAST_EXEC_NS, LAST_RESULTS
    state = np.ascontiguousarray(np.asarray(state, dtype=np.float32))
    accumulator = np.ascontiguousarray(np.asarray(accumulator, dtype=np.float32))

    wq_t = np.ascontiguousarray(np.asarray(Wq, np.float32).T).astype(NPBF16)
    wk_t = np.ascontiguousarray(np.asarray(Wk, np.float32).T).astype(NPBF16)
    wv_t = (np.ascontiguousarray(np.asarray(Wv, np.float32).T) / S).astype(NPBF16)
    wo_t = (np.ascontiguousarray(np.asarray(Wo, np.float32).T)
            / np.sqrt(D)).astype(NPBF16)

    in_maps = []
    for c in range(N_CORES):
        b, h = c // 2, c % 2
        in_maps.append({
            "x": np.ascontiguousarray(state[b, h * S_LOC:(h + 1) * S_LOC]),
            "wq_t": wq_t, "wk_t": wk_t, "wv_t": wv_t, "wo_t": wo_t,
            "acc_t": np.ascontiguousarray(accumulator[b].T),
        })

    nc = _get_nc()
    res = bass_utils.run_bass_kernel_spmd(
        nc, in_maps, core_ids=list(range(N_CORES)), trace=TRACE)
    LAST_EXEC_NS = res.exec_time_ns
    LAST_RESULTS = res

    out = np.empty((B, S, D), dtype=np.float32)
    acc = np.empty((B, D, D), dtype=np.float32)
    for c in range(N_CORES):
        b, h = c // 2, c % 2
        out[b, h * S_LOC:(h + 1) * S_LOC] = res.results[c]["out"]
        if h == 0:
            acc[b] = res.results[c]["acc_o"].T
    return out, acc


# revision 10
# speedup vs baseline: 1.0491x; 1.0212x over previous
"""DeepFloorFusedMixer Trainium2 kernel.

Math (per batch b):
    normed = rms_norm(state)
    q = phi(normed @ Wq^T); k = phi(normed @ Wk^T); v = normed @ Wv^T
    acc = 0.95*acc_in + (k^T v)/S
    out = state + (q @ acc)/sqrt(D) @ Wo^T

Algebraic restructure used on device:
    G^T = (nb^T phi_k)^T  accumulated over sequence (per core: its half)
    kv^T = (Wv^T/S)^T-contract G^T      (v never materialized)
    accT = 0.95*accT_in + kvT           (transposed accumulator)
    M = accT^T-contract (Wo^T/sqrt(D))  -> out = state + qT^T-contract M

Sharding: core c handles batch c//2, sequence half c%2 (4096 rows).
The per-pair [D,D] G^T partial sums are combined with a 2-core AllReduce.
All matmuls run in bf16 with fp32 PSUM accumulation.
"""

import sys
import types
import numpy as np

sys.path.insert(0, "/opt/trn_rl_repo")


def _install_ntff_shim():
    """Register the axon NTFF profiling hook (needed only when TRACE=True)."""
    try:
        import antenv
        if hasattr(antenv, "axon_hooks"):
            return
        sys.path.insert(0, "/root/.axon_site/trn_agent_boot")
        import trn_boot
        hook = trn_boot._ntff_profile_via_ctypes("/opt/axon/libaxon_pjrt.so")
        mod = types.ModuleType("antenv.axon_hooks")
        mod.get_axon_ntff_profile_hook = lambda: hook
        mod.set_axon_ntff_profile_hook = lambda h: None
        sys.modules["antenv.axon_hooks"] = mod
        antenv.axon_hooks = mod
    except Exception:
        pass

import ml_dtypes  # noqa: E402
import concourse.bass as bass  # noqa: E402
import concourse.bacc as bacc  # noqa: E402
import concourse.tile as tile  # noqa: E402
import concourse.mybir as mybir  # noqa: E402
from concourse import bass_utils, masks  # noqa: E402

F32 = mybir.dt.float32
BF16 = mybir.dt.bfloat16
NPBF16 = ml_dtypes.bfloat16

B, S, D = 4, 8192, 1024
N_CORES = 8
S_LOC = S // 2          # rows per core
P = 128                 # partitions
NT = D // P             # 8 feature tiles
N_BLK = 4               # blocks per core
BLK_ST = S_LOC // (N_BLK * P)   # 8 s_tiles per block
BLK_S = BLK_ST * P      # 1024 rows per block
DECAY = 0.95
EPS = 1e-6

# module-level cache: build+compile once per process
_CACHE = {}

# set by callers that want profiling (see test.py)
TRACE = False
LAST_EXEC_NS = None
LAST_RESULTS = None


def _build():
    nc = bacc.Bacc("TRN2", target_bir_lowering=False, debug=False,
                   num_devices=N_CORES)

    x_d = nc.dram_tensor("x", [S_LOC, D], F32, kind="ExternalInput")
    wk_d = nc.dram_tensor("wk_t", [D, D], BF16, kind="ExternalInput")
    wq_d = nc.dram_tensor("wq_t", [D, D], BF16, kind="ExternalInput")
    wv_d = nc.dram_tensor("wv_t", [D, D], BF16, kind="ExternalInput")
    wo_d = nc.dram_tensor("wo_t", [D, D], BF16, kind="ExternalInput")
    acc_d = nc.dram_tensor("acc_t", [D, D], F32, kind="ExternalInput")
    out_d = nc.dram_tensor("out", [S_LOC, D], F32, kind="ExternalOutput")
    acco_d = nc.dram_tensor("acc_o", [D, D], F32, kind="ExternalOutput")

    x_v = x_d.ap().rearrange("(n p) d -> n p d", p=P)      # [32, 128, 1024]
    out_v = out_d.ap().rearrange("(n p) d -> n p d", p=P)
    acc_v = acc_d.ap().rearrange("(n p) d -> n p d", p=P)  # [8, 128, 1024]
    acco_v = acco_d.ap().rearrange("(n p) d -> n p d", p=P)
    # weights viewed as [tile, 128, 1024]
    wk_v = wk_d.ap().rearrange("(n p) d -> p n d", p=P)    # [128, 8, 1024]
    wq_v = wq_d.ap().rearrange("(n p) d -> p n d", p=P)
    wv_v = wv_d.ap().rearrange("(n p) d -> p n d", p=P)
    wo_v = wo_d.ap().rearrange("(n p) d -> p n d", p=P)

    with tile.TileContext(nc) as tc:
        with (
            tc.tile_pool(name="const", bufs=1) as constp,
            tc.tile_pool(name="wpool", bufs=2) as wpool,
            tc.tile_pool(name="gpool", bufs=1) as gpool,
            tc.tile_pool(name="btpool", bufs=5) as btpool,
            tc.tile_pool(name="xpool", bufs=2) as xpool,
            tc.tile_pool(name="opool", bufs=2) as opool,
            tc.tile_pool(name="sqpool", bufs=1) as sqpool,
            tc.tile_pool(name="stpool", bufs=4) as stpool,
            tc.tile_pool(name="phpool", bufs=2) as phpool,
            tc.tile_pool(name="qtpool", bufs=2) as qtpool,
            tc.tile_pool(name="acpool", bufs=2) as acpool,
            tc.tile_pool(name="perm", bufs=1) as permp,
            tc.tile_pool(name="mmps", bufs=3, space="PSUM") as mmps,
            tc.tile_pool(name="trps", bufs=2, space="PSUM") as trps,
            tc.tile_pool(name="dram", bufs=1, space="DRAM") as dramp,
        ):
            ident = constp.tile([P, P], BF16)
            masks.make_identity(nc, ident[:])
            eps_t = constp.tile([P, 1], F32)
            nc.gpsimd.memset(eps_t[:], EPS)

            wk_sb = wpool.tile([P, NT, D], BF16, name="wk_sb", tag="w")
            wq_sb = wpool.tile([P, NT, D], BF16, name="wq_sb", tag="w")
            nc.sync.dma_start(wk_sb[:], wk_v)
            nc.sync.dma_start(wq_sb[:], wq_v)

            gt_sbuf = gpool.tile([P, NT, D], BF16, name="gt_sbuf", tag="gt")
            acct_bf = permp.tile([P, NT, D], BF16, name="acct_bf", tag="acct")
            m_sb = permp.tile([P, NT, D], BF16, name="m_sb", tag="m")

            qt_spill = dramp.tile([N_BLK, NT, P, BLK_S], BF16, name="qt_spill", tag="qt_spill")
            gt_loc = dramp.tile([P, NT, D], BF16, name="gt_loc", tag="gt_loc")
            ag_out = dramp.tile([2, P, NT, D], BF16, name="ag_out", tag="ag_out")

            deferred_nbt = {}
            # ---------------- block loop ----------------
            for blk in range(N_BLK):
                nb_blk = btpool.tile([P, BLK_ST, D], BF16, name=f"nb{blk}", tag="bt")
                nbt_blk = btpool.tile([P, NT, BLK_S], BF16, name=f"nbt{blk}", tag="bt")
                phik_blk = btpool.tile([P, BLK_ST, D], BF16, name=f"phik{blk}", tag="bt")

                for sl in range(BLK_ST):
                    st = blk * BLK_ST + sl
                    # --- rmsnorm ---
                    x_t = xpool.tile([P, D], F32, name="x_t", tag="x_t")
                    nc.sync.dma_start(x_t[:], x_v[st])
                    sq = sqpool.tile([P, D], BF16, name="sq", tag="sq")
                    ss = stpool.tile([P, 1], F32, name="ss", tag="ss")
                    nc.scalar.activation(
                        sq[:], x_t[:], mybir.ActivationFunctionType.Square,
                        accum_out=ss[:],
                    )
                    sr = stpool.tile([P, 1], F32, name="sr", tag="sr")
                    nc.scalar.activation(
                        sr[:], ss[:], mybir.ActivationFunctionType.Sqrt,
                        bias=eps_t[:], scale=1.0 / D,
                    )
                    rstd = stpool.tile([P, 1], F32, name="rstd", tag="rstd")
                    nc.vector.reciprocal(rstd[:], sr[:])
                    nc.vector.tensor_scalar_mul(
                        nb_blk[:, sl, :], x_t[:], rstd[:])

                    # --- transpose nb tile -> nbt (one psum group per bank) ---
                    tr = trps.tile([P, D], BF16, name="tr", tag="tr")
                    for dt in range(NT):
                        nc.tensor.matmul(
                            tr[:, dt * P:(dt + 1) * P],
                            nb_blk[:, sl, dt * P:(dt + 1) * P],
                            ident[:],
                            is_transpose=True,
                            start=(dt == 0), stop=(dt == NT - 1),
                        )
                    # scatter psum -> nbt_blk[:, :, sl*P : (sl+1)*P]
                    nc.scalar.copy(
                        nbt_blk[:, :, sl * P:(sl + 1) * P],
                        tr[:].rearrange("p (n f) -> p n f", f=P),
                    )

                    # --- k projection ---
                    kp = mmps.tile([P, D], F32, name="kp", tag="mm")
                    for dt in range(NT):
                        for hf in range(2):
                            nc.tensor.matmul(
                                kp[:, hf * 512:(hf + 1) * 512],
                                nbt_blk[:, dt, sl * P:(sl + 1) * P],
                                wk_sb[:, dt, hf * 512:(hf + 1) * 512],
                                start=(dt == 0), stop=(dt == NT - 1),
                            )
                    # --- phi(k) = max(x+1, min(exp(x), 1)) ---
                    ex = phpool.tile([P, D], BF16, name="ex", tag="ex")
                    nc.scalar.activation(
                        ex[:], kp[:], mybir.ActivationFunctionType.Exp)
                    xp1 = phpool.tile([P, D], BF16, name="xp1", tag="xp1")
                    nc.vector.tensor_scalar_add(xp1[:], kp[:], 1.0)
                    nc.vector.tensor_scalar_min(ex[:], ex[:], 1.0)
                    nc.vector.tensor_tensor(
                        phik_blk[:, sl, :], ex[:], xp1[:],
                        op=mybir.AluOpType.max)

                # --- G^T accumulation: GT[g,d] += sum_s nb[s,g] phik[s,d] ---
                for gt in range(NT):
                    gp = mmps.tile([P, D], F32, name="gp", tag="mm")
                    for sl in range(BLK_ST):
                        for hf in range(2):
                            nc.tensor.matmul(
                                gp[:, hf * 512:(hf + 1) * 512],
                                nb_blk[:, sl, gt * P:(gt + 1) * P],
                                phik_blk[:, sl, hf * 512:(hf + 1) * 512],
                                start=(sl == 0), stop=(sl == BLK_ST - 1),
                            )
                    if blk == 0:
                        nc.vector.tensor_copy(gt_sbuf[:, gt, :], gp[:])
                    else:
                        nc.vector.tensor_tensor(
                            gt_sbuf[:, gt, :], gp[:], gt_sbuf[:, gt, :],
                            op=mybir.AluOpType.add)
                    if blk == N_BLK - 1:
                        nc.sync.dma_start(gt_loc[:, gt, :], gt_sbuf[:, gt, :])

                # --- qT production (deferred for the last block so it
                #     overlaps the collective) ---
                def emit_qt(qblk, qnbt):
                    for et in range(NT):
                        qp = mmps.tile([P, BLK_S], F32, name="qp", tag="mm")
                        for dt in range(NT):
                            for hf in range(2):
                                nc.tensor.matmul(
                                    qp[:, hf * 512:(hf + 1) * 512],
                                    wq_sb[:, dt, et * P:(et + 1) * P],
                                    qnbt[:, dt, hf * 512:(hf + 1) * 512],
                                    start=(dt == 0), stop=(dt == NT - 1),
                                )
                        exq = phpool.tile([P, BLK_S], BF16, name="exq", tag="ex")
                        nc.scalar.activation(
                            exq[:], qp[:], mybir.ActivationFunctionType.Exp)
                        xp1q = phpool.tile([P, BLK_S], BF16, name="xp1q", tag="xp1")
                        nc.vector.tensor_scalar_add(xp1q[:], qp[:], 1.0)
                        nc.vector.tensor_scalar_min(exq[:], exq[:], 1.0)
                        qt_t = qtpool.tile([P, BLK_S], BF16, name="qt_t", tag="qt")
                        nc.vector.tensor_tensor(
                            qt_t[:], exq[:], xp1q[:], op=mybir.AluOpType.max)
                        nc.sync.dma_start(qt_spill[qblk, et], qt_t[:])

                if blk < N_BLK - 2:
                    emit_qt(blk, nbt_blk)
                else:
                    deferred_nbt[blk] = nbt_blk

            # ------- pairwise exchange of G^T (AllGather + local sum) -------
            nc.gpsimd.collective_compute(
                "AllGather",
                mybir.AluOpType.bypass,
                replica_groups=[[0, 1], [2, 3], [4, 5], [6, 7]],
                ins=[gt_loc[:].opt()],
                outs=[ag_out[:].opt()],
            )
            # deferred qT of the last two blocks runs on PE while the
            # collective is in flight
            emit_qt(N_BLK - 2, deferred_nbt[N_BLK - 2])
            emit_qt(N_BLK - 1, deferred_nbt[N_BLK - 1])
            ag0 = btpool.tile([P, NT, D], BF16, name="ag0", tag="bt")
            ag1 = btpool.tile([P, NT, D], BF16, name="ag1", tag="bt")
            for gt in range(NT):
                nc.sync.dma_start(ag0[:, gt, :], ag_out[0, :, gt, :])
                nc.sync.dma_start(ag1[:, gt, :], ag_out[1, :, gt, :])
                nc.vector.tensor_tensor(
                    gt_sbuf[:, gt, :], ag0[:, gt, :], ag1[:, gt, :],
                    op=mybir.AluOpType.add)

            # ---------------- kv^T and accT ----------------
            wv_sb = wpool.tile([P, NT, D], BF16, name="wv_sb", tag="w")
            nc.sync.dma_start(wv_sb[:], wv_v)
            for et in range(NT):
                kvp = mmps.tile([P, D], F32, name="kvp", tag="mm")
                for gt in range(NT):
                    for hf in range(2):
                        nc.tensor.matmul(
                            kvp[:, hf * 512:(hf + 1) * 512],
                            wv_sb[:, gt, et * P:(et + 1) * P],
                            gt_sbuf[:, gt, hf * 512:(hf + 1) * 512],
                            start=(gt == 0), stop=(gt == NT - 1),
                        )
                a_in = acpool.tile([P, D], F32, name="a_in", tag="a_in")
                nc.sync.dma_start(a_in[:], acc_v[et])
                nc.vector.tensor_scalar_mul(a_in[:], a_in[:], DECAY)
                a_new = acpool.tile([P, D], F32, name="a_new", tag="a_new")
                nc.vector.tensor_tensor(
                    a_new[:], kvp[:], a_in[:], op=mybir.AluOpType.add)
                nc.sync.dma_start(acco_v[et], a_new[:])
                nc.vector.tensor_copy(acct_bf[:, et, :], a_new[:])

            # ---------------- M = accT.T-contract WoT ----------------
            wo_sb = wpool.tile([P, NT, D], BF16, name="wo_sb", tag="w")
            nc.sync.dma_start(wo_sb[:], wo_v)
            for dt in range(NT):
                mp = mmps.tile([P, D], F32, name="mp", tag="mm")
                for et in range(NT):
                    for hf in range(2):
                        nc.tensor.matmul(
                            mp[:, hf * 512:(hf + 1) * 512],
                            acct_bf[:, et, dt * P:(dt + 1) * P],
                            wo_sb[:, et, hf * 512:(hf + 1) * 512],
                            start=(et == 0), stop=(et == NT - 1),
                        )
                nc.scalar.copy(m_sb[:, dt, :], mp[:])

            # ---------------- out = x + qT.T-contract M ----------------
            for blk in range(N_BLK):
                qt_blk = btpool.tile([P, NT, BLK_S], BF16, name=f"qtb{blk}", tag="bt")
                nc.sync.dma_start(qt_blk[:], qt_spill[blk].rearrange("n p f -> p n f"))
                for sl in range(BLK_ST):
                    st = blk * BLK_ST + sl
                    xo_t = xpool.tile([P, D], F32, name="xo_t", tag="x_t")
                    nc.sync.dma_start(xo_t[:], x_v[st])
                    op_ps = mmps.tile([P, D], F32, name="op_ps", tag="mm")
                    for hf in range(2):
                        for dt in range(NT):
                            nc.tensor.matmul(
                                op_ps[:, hf * 512:(hf + 1) * 512],
                                qt_blk[:, dt, sl * P:(sl + 1) * P],
                                m_sb[:, dt, hf * 512:(hf + 1) * 512],
                                start=(dt == 0), stop=(dt == NT - 1),
                            )
                    o_t = opool.tile([P, D], F32, name="o_t", tag="o_t")
                    nc.vector.tensor_tensor(
                        o_t[:], op_ps[:], xo_t[:], op=mybir.AluOpType.add)
                    nc.sync.dma_start(out_v[st], o_t[:])

    nc.compile()
    return nc


def _get_nc():
    if "nc" not in _CACHE:
        _CACHE["nc"] = _build()
    return _CACHE["nc"]


def kernel(state, accumulator, Wq, Wk, Wv, Wo):
    global LAST_EXEC_NS, LAST_RESULTS
    state = np.ascontiguousarray(np.asarray(state, dtype=np.float32))
    accumulator = np.ascontiguousarray(np.asarray(accumulator, dtype=np.float32))

    wq_t = np.ascontiguousarray(np.asarray(Wq, np.float32).T).astype(NPBF16)
    wk_t = np.ascontiguousarray(np.asarray(Wk, np.float32).T).astype(NPBF16)
    wv_t = (np.ascontiguousarray(np.asarray(Wv, np.float32).T) / S).astype(NPBF16)
    wo_t = (np.ascontiguousarray(np.asarray(Wo, np.float32).T)
            / np.sqrt(D)).astype(NPBF16)

    in_maps = []
    for c in range(N_CORES):
        b, h = c // 2, c % 2
        in_maps.append({
            "x": np.ascontiguousarray(state[b, h * S_LOC:(h + 1) * S_LOC]),
            "wq_t": wq_t, "wk_t": wk_t, "wv_t": wv_t, "wo_t": wo_t,
            "acc_t": np.ascontiguousarray(accumulator[b].T),
        })

    if TRACE:
        _install_ntff_shim()
    nc = _get_nc()
    res = bass_utils.run_bass_kernel_spmd(
        nc, in_maps, core_ids=list(range(N_CORES)), trace=TRACE)
    LAST_EXEC_NS = res.exec_time_ns
    LAST_RESULTS = res

    out = np.empty((B, S, D), dtype=np.float32)
    acc = np.empty((B, D, D), dtype=np.float32)
    for c in range(N_CORES):
        b, h = c // 2, c % 2
        out[b, h * S_LOC:(h + 1) * S_LOC] = res.results[c]["out"]
        if h == 0:
            acc[b] = res.results[c]["acc_o"].T
    return out, acc
